# revision 1
# baseline (speedup 1.0000x reference)
"""Trainium2 Bass kernel: 3x3 stride-1 pad-1 conv2d, N=16,Cin=64,Cout=128,H=W=224.

Sharding: data-parallel over batch: 8 cores x 2 images each.

Per-core algorithm:
  - x lives in SBUF bands of R output rows per image at W+1=225 row stride:
    band row i = image row y0-1+i; element 224 of each row is a zero pad
    column (plus a zero guard element before row 0). With flat-shifted
    taps, out(y,0)'s dc=0 tap reads the previous row's pad and
    out(y,223)'s dc=2 tap reads its own row's pad -- both zero, so border
    columns come out exactly right with no fix-up pass.
    partitions 0-63 = img0 channels, 64-127 = img1 channels.
  - conv = sum over 9 taps (dr,dc) of fp16 matmuls:
      psum[co, 2 rows] += w[ci, tap, co].T @ band[ci, rows 2c+dr.., cols dc-1..]
    K=64 (Cin) partitions, M=128 (Cout), N=448 (2 output rows, one PSUM
    bank). fp16 in, fp22 multiply, fp32 accumulate; x is cast to fp16 on
    the host (halves input HBM traffic), weights are fp16.
  - img0 matmuls use PE rows 0-63, img1 rows 64-127 (tile_position derived
    from base partitions) -> the two streams run concurrently in disjoint
    row-groups of the systolic array (~107 ns per matmul sustained).
  - PSUM chunk [128, 448] evicted to SBUF staging with fused bias add
    (DVE 3/4, ACT 1/4); staged band DMA'd out on the scalar-engine queue
    so input loads (sync queue) and output stores overlap.
"""

import numpy as np

N_IMG, C_IN, C_OUT, KS, H, W = 16, 64, 128, 3, 224, 224
N_CORES = 8
IMGS_PER_CORE = N_IMG // N_CORES  # 2
R = 32  # output rows per band
WP = W + 1  # padded row stride in SBUF
TAPS = [(dr, dc) for dr in range(KS) for dc in range(KS)]


def build_conv_program(h=H, w=W, r=R, evict_split=3, out_bf16=False):
    import concourse.bacc as bacc
    import concourse.mybir as mybir
    import concourse.tile as tile

    wp = w + 1
    n_bands = h // r
    n_chunk = r // 2
    chunk = 2 * w  # 448
    flat = r * w
    assert h % r == 0 and r % 2 == 0
    # 1 guard elem (acts as row -1's pad), (r+2) rows of wp, 1 tail guard
    band_len = 1 + (r + 2) * wp + 1
    f32 = mybir.dt.float32
    f16 = mybir.dt.float16
    odt = mybir.dt.bfloat16 if out_bf16 else f32

    nc = bacc.Bacc("TRN2", target_bir_lowering=False)

    x_d = nc.dram_tensor("x", [IMGS_PER_CORE, C_IN, h, w], f16, kind="ExternalInput")
    w_d = nc.dram_tensor("w", [C_IN, 9, C_OUT], f16, kind="ExternalInput")
    b_d = nc.dram_tensor("bias", [C_OUT, 1], f32, kind="ExternalInput")
    zz_d = nc.dram_tensor("zz", [128, w], f16, kind="ExternalInput")
    out_d = nc.dram_tensor(
        "out", [IMGS_PER_CORE, C_OUT, h, w], odt, kind="ExternalOutput"
    )

    with tile.TileContext(nc) as tc:
        with (
            tc.tile_pool(name="const", bufs=1) as const_pool,
            tc.tile_pool(name="xband", bufs=2) as x_pool,
            tc.tile_pool(name="outs", bufs=2) as o_pool,
            tc.tile_pool(name="psum", bufs=8, space="PSUM") as p_pool,
        ):
            # fp16 weights: the per-matmul LDWEIGHTS hides under the N=448
            # moving stream. PE upconverts to fp22; accumulation is fp32.
            w_sb = const_pool.tile([128, 9, C_OUT], f16)
            nc.sync.dma_start(out=w_sb[0:64], in_=w_d[:])
            nc.sync.dma_start(out=w_sb[64:128], in_=w_d[:])
            bias_sb = const_pool.tile([C_OUT, 1], f32)
            nc.sync.dma_start(out=bias_sb[:], in_=b_d[:])

            bands = [
                x_pool.tile([128, band_len], f16, tag="band", name=f"band{i}")
                for i in range(2)
            ]
            for bt in bands:
                # zero the pad column of every row + the leading guard elem;
                # interior DMAs never touch these, so once is enough.
                nc.vector.memset(bt[:, 0 : 1 + (r + 2) * wp : wp], 0.0)

            for b in range(n_bands):
                y0 = b * r
                bt = bands[b % 2]
                bv = bt[:, 1 : 1 + (r + 2) * wp].rearrange(
                    "p (a c) -> p a c", c=wp
                )
                rows_lo = max(y0 - 1, 0)
                rows_hi = min(y0 + r + 1, h)
                dst_r0 = rows_lo - (y0 - 1)
                nrows = rows_hi - rows_lo
                if b == 0:
                    # top halo row of the image is zero
                    nc.sync.dma_start(out=bv[:, 0, 0:w], in_=zz_d[:])
                if b == n_bands - 1:
                    # bottom halo row is zero (buffer may hold stale data)
                    nc.sync.dma_start(out=bv[:, r + 1, 0:w], in_=zz_d[:])
                n_pieces = 4 if b == 0 else 1
                for img in range(IMGS_PER_CORE):
                    p0 = img * 64
                    for pc in range(n_pieces):
                        r_a = pc * nrows // n_pieces
                        r_b = (pc + 1) * nrows // n_pieces
                        nc.sync.dma_start(
                            out=bv[p0 : p0 + 64, dst_r0 + r_a : dst_r0 + r_b, 0:w],
                            in_=x_d[img, :, rows_lo + r_a : rows_lo + r_b, :],
                        )

                ost = [
                    o_pool.tile(
                        [C_OUT, flat], odt, tag=f"ost{img}", name=f"ost{img}_{b}"
                    )
                    for img in range(IMGS_PER_CORE)
                ]

                for c in range(n_chunk):
                    ps = [
                        p_pool.tile(
                            [C_OUT, chunk],
                            f32,
                            tag="ps",
                            bufs=8,
                            name=f"ps{i}_{b}_{c}",
                        )
                        for i in range(2)
                    ]
                    for t, (dr, dc) in enumerate(TAPS):
                        st = t == 0
                        sp = t == 8
                        base = 1 + (2 * c + dr) * wp + dc - 1
                        for img in range(IMGS_PER_CORE):
                            p0 = img * 64
                            rhs = bt[p0 : p0 + 64, base : base + 2 * wp].rearrange(
                                "p (a c) -> p a c", c=wp
                            )[:, :, 0:w]
                            nc.tensor.matmul(
                                ps[img][:],
                                w_sb[p0 : p0 + 64, t, :],
                                rhs,
                                start=st,
                                stop=sp,
                            )
                    for img in range(IMGS_PER_CORE):
                        dst = ost[img][:, c * chunk : (c + 1) * chunk]
                        if (c % 4) < evict_split:
                            nc.vector.tensor_scalar_add(dst, ps[img][:], bias_sb[:])
                        else:
                            nc.scalar.add(dst, ps[img][:], bias_sb[:])

                # Split stores so the final piece overlaps trailing evictions.
                n_out = 2
                for img in range(IMGS_PER_CORE):
                    for oc in range(n_out):
                        r_a = oc * r // n_out
                        r_b = (oc + 1) * r // n_out
                        nc.scalar.dma_start(
                            out=out_d[img, :, y0 + r_a : y0 + r_b, :],
                            in_=ost[img][:, r_a * w : r_b * w],
                        )

    nc.compile()
    return nc


def prep_weight(weight: np.ndarray) -> np.ndarray:
    # [C_OUT, C_IN, 3, 3] -> [C_IN, 9, C_OUT]
    return np.ascontiguousarray(weight.transpose(1, 2, 3, 0).reshape(C_IN, 9, C_OUT))


def run_conv(x, weight, bias, trace=False, h=H, r=R, out_bf16=False, evict_split=3):
    """x [16,64,224,224] f32. Returns (out [16,128,224,224] f32, results)."""
    from concourse.bass_utils import run_bass_kernel_spmd

    x = np.asarray(x, dtype=np.float32).astype(np.float16)
    w_t = prep_weight(np.asarray(weight, dtype=np.float32)).astype(np.float16)
    b_t = np.ascontiguousarray(np.asarray(bias, dtype=np.float32).reshape(C_OUT, 1))

    nc = build_conv_program(h=h, r=r, out_bf16=out_bf16, evict_split=evict_split)
    zz_np = np.zeros((128, W), np.float16)
    in_maps = [
        {
            "x": np.ascontiguousarray(x[i * IMGS_PER_CORE : (i + 1) * IMGS_PER_CORE]),
            "w": w_t,
            "bias": b_t,
            "zz": zz_np,
        }
        for i in range(N_CORES)
    ]
    res = run_bass_kernel_spmd(nc, in_maps, core_ids=list(range(N_CORES)), trace=trace)
    out = np.concatenate([r_["out"] for r_ in res.results], axis=0)
    if out.dtype != np.float32:
        out = out.astype(np.float32)
    return out, res


def kernel(**inputs) -> np.ndarray:
    out, _ = run_conv(inputs["x"], inputs["weight"], inputs["bias"])
    return out



# revision 3
# speedup vs baseline: 1.2818x; 1.2818x over previous
"""Trainium2 Bass kernel: 3x3 stride-1 pad-1 conv2d, N=16,Cin=64,Cout=128,H=W=224.

Sharding: data-parallel over batch: 8 cores x 2 images each.

v2: contiguous input slabs (big DMA descriptors) + PE border-fix.

Per-core algorithm:
  - x lives in SBUF slabs of 34 rows per image at 224 stride (fully
    contiguous per partition -> one ~15KB DMA descriptor per partition
    instead of 34 448-byte row writes; input DMA runs at wire speed).
    partitions 0-63 = img0 channels, 64-127 = img1 channels. One guard
    element before row 0 and after row 33 absorbs the flat-shift
    over/underflow of the dc=0/dc=2 taps.
  - conv = sum over 9 taps (dr,dc) of fp16 matmuls:
      psum[co, 2 rows] += w[ci, tap, co].T @ slab[ci, rows 2c+dr.., cols dc-1..]
    K=64 (Cin), M=128 (Cout), N=448 (2 output rows, one PSUM bank).
    img0 uses PE rows 0-63, img1 rows 64-127 (concurrent row groups).
  - Without a pad column, out(:, 0) picks up a wrap-around term
    w[dr,0]*x(row-1, 223) and out(:, 223) picks up w[dr,2]*x(row+1, 0).
    Fix: 3+3 tiny K=64 N=32 matmuls per image per band compute the
    negated wrap terms (reading the exact same SBUF addresses the main
    pass read, so cancellation is exact even for the guard elements)
    into a PSUM strip; a strided DVE tensor_add patches columns 0/223
    of the staged output. Costs ~2us of PE vs ~70us of DMA overhead.
  - PSUM chunk [128, 448] evicted to SBUF staging with fused bias add
    (DVE 3/4, ACT 1/4); output staged in bf16 (halves store traffic),
    stores on the scalar-engine queue so loads (sync queue) overlap.
"""

import numpy as np

N_IMG, C_IN, C_OUT, KS, H, W = 16, 64, 128, 3, 224, 224
N_CORES = 8
IMGS_PER_CORE = N_IMG // N_CORES  # 2
R = 32  # output rows per band
TAPS = [(dr, dc) for dr in range(KS) for dc in range(KS)]


def build_conv_program(h=H, w=W, r=R, evict_split=3, out_bf16=True):
    import concourse.bacc as bacc
    import concourse.mybir as mybir
    import concourse.tile as tile

    n_bands = h // r
    n_chunk = r // 2
    chunk = 2 * w  # 448
    flat = r * w  # 7168
    assert h % r == 0 and r % 2 == 0
    nrow_slab = r + 2  # 34
    # guard elem + 34 contiguous rows + tail guard elem + pad so the
    # border-fix rhs rearrange windows stay in bounds (never read)
    slab_len = 1 + nrow_slab * w + 1 + 2 * w
    f32 = mybir.dt.float32
    f16 = mybir.dt.float16
    odt = mybir.dt.bfloat16 if out_bf16 else f32

    nc = bacc.Bacc("TRN2", target_bir_lowering=False)

    x_d = nc.dram_tensor("x", [IMGS_PER_CORE, C_IN, h, w], f16, kind="ExternalInput")
    w_d = nc.dram_tensor("w", [C_IN, 9, C_OUT], f16, kind="ExternalInput")
    wn_d = nc.dram_tensor("wneg", [C_IN, 6, C_OUT], f16, kind="ExternalInput")
    b_d = nc.dram_tensor("bias", [C_OUT, 1], f32, kind="ExternalInput")
    out_d = nc.dram_tensor(
        "out", [IMGS_PER_CORE, C_OUT, h, w], odt, kind="ExternalOutput"
    )

    with tile.TileContext(nc) as tc:
        with (
            tc.tile_pool(name="const", bufs=1) as const_pool,
            tc.tile_pool(name="xslab", bufs=2) as x_pool,
            tc.tile_pool(name="outs", bufs=2) as o_pool,
            tc.tile_pool(name="psum", bufs=8, space="PSUM") as p_pool,
        ):
            # fp16 weights: per-matmul LDWEIGHTS hides under the N=448
            # moving stream. PE upconverts to fp22; accumulation is fp32.
            w_sb = const_pool.tile([128, 9, C_OUT], f16)
            nc.sync.dma_start(out=w_sb[0:64], in_=w_d[:])
            nc.sync.dma_start(out=w_sb[64:128], in_=w_d[:])
            # negated border taps: [dr] = -w[dr,0], [3+dr] = -w[dr,2].
            # Their DMAs are issued after band 0's first pieces (below) so
            # the first chunk's data lands earlier in the queue.
            wn_sb = const_pool.tile([128, 6, C_OUT], f16)
            bias_sb = const_pool.tile([C_OUT, 1], f32)

            # HAM warm-up: keep the PE busy while the first band loads so the
            # clock gate is open when real matmuls start. Reads the weight
            # tile (first DMA to land); the result is never consumed.
            warm = p_pool.tile([C_OUT, chunk], f32, tag="ps", bufs=6, name="warm")
            for _ in range(8):
                nc.tensor.matmul(
                    warm[:, 0 : 3 * C_OUT],
                    w_sb[0:64, 0, :],
                    w_sb[0:64, 0:3, :],
                    start=True,
                    stop=True,
                )

            slabs = [
                x_pool.tile([128, slab_len], f16, tag="slab", name=f"slab{i}")
                for i in range(2)
            ]
            for st in slabs:
                # zero both guard elements once; loads never touch them.
                nc.vector.memset(st[:, 0:1], 0.0)
                nc.vector.memset(st[:, slab_len - 1 : slab_len], 0.0)
            # top halo row of the image is zero (band 0 loads rows 1..33)
            nc.vector.memset(slabs[0][:, 1 : 1 + w], 0.0)

            for b in range(n_bands):
                y0 = b * r
                bt = slabs[b % 2]
                rows_lo = max(y0 - 1, 0)
                rows_hi = min(y0 + r + 1, h)
                dst_r0 = rows_lo - (y0 - 1)
                nrows = rows_hi - rows_lo
                if b == n_bands - 1:
                    # bottom halo row is zero (buffer holds stale data)
                    nc.vector.memset(bt[:, 1 + (r + 1) * w : 1 + (r + 2) * w], 0.0)
                # img-interleaved pieces so both images' first rows land
                # early (first chunk needs rows 0..3 of BOTH images)
                n_pieces = 8 if b == 0 else 1
                for pc in range(n_pieces):
                    r_a = pc * nrows // n_pieces
                    r_b = (pc + 1) * nrows // n_pieces
                    for img in range(IMGS_PER_CORE):
                        p0 = img * 64
                        nc.sync.dma_start(
                            out=bt[
                                p0 : p0 + 64,
                                1 + (dst_r0 + r_a) * w : 1 + (dst_r0 + r_b) * w,
                            ],
                            in_=x_d[img, :, rows_lo + r_a : rows_lo + r_b, :],
                        )
                    if b == 0 and pc == 1:
                        nc.sync.dma_start(out=bias_sb[:], in_=b_d[:])
                        nc.sync.dma_start(out=wn_sb[0:64], in_=wn_d[:])
                        nc.sync.dma_start(out=wn_sb[64:128], in_=wn_d[:])

                ost = [
                    o_pool.tile(
                        [C_OUT, flat], odt, tag=f"ost{img}", name=f"ost{img}_{b}"
                    )
                    for img in range(IMGS_PER_CORE)
                ]

                # Border wrap-around fix: strip[img][:, 0:32] = F (col 0),
                # strip[img][:, 32:64] = G (col 223). Each is the negated
                # sum over dr of the wrong term the main pass accumulated,
                # read from the identical SBUF addresses. One PSUM bank per
                # image so the two row-group streams never share a bank.
                # Needs only the slab, so on prefetched bands (b>0) it is
                # issued BEFORE the main matmuls: strips are ready early and
                # the last band's patches/stores pipeline with the trailing
                # evictions. On band 0 this order would stall the in-order
                # PE queue on the full band load, so it goes last there.
                def emit_fix(bt=bt, b=b):
                    strips = [
                        p_pool.tile(
                            [C_OUT, 64],
                            f32,
                            tag=f"strip{i}",
                            bufs=1,
                            name=f"strip{i}_{b}",
                        )
                        for i in range(IMGS_PER_CORE)
                    ]
                    for img in range(IMGS_PER_CORE):
                        p0 = img * 64
                        for dr in range(KS):
                            # F: main dc=0 tap read flat[(rho+dr)*w - 1 + 1]
                            off = dr * w
                            rhs = bt[p0 : p0 + 64, off : off + r * w].rearrange(
                                "p (a c) -> p a c", c=w
                            )[:, :, 0:1]
                            nc.tensor.matmul(
                                strips[img][:, 0:r],
                                wn_sb[p0 : p0 + 64, dr, :],
                                rhs,
                                start=(dr == 0),
                                stop=(dr == KS - 1),
                            )
                        for dr in range(KS):
                            # G: main dc=2 tap read flat[1 + (rho+dr+1)*w]
                            off = 1 + (dr + 1) * w
                            rhs = bt[p0 : p0 + 64, off : off + r * w].rearrange(
                                "p (a c) -> p a c", c=w
                            )[:, :, 0:1]
                            nc.tensor.matmul(
                                strips[img][:, r : 2 * r],
                                wn_sb[p0 : p0 + 64, 3 + dr, :],
                                rhs,
                                start=(dr == 0),
                                stop=(dr == KS - 1),
                            )
                    return strips

                strips = None

                for c in range(n_chunk):
                    ps = [
                        p_pool.tile(
                            [C_OUT, chunk],
                            f32,
                            tag="ps",
                            bufs=6,
                            name=f"ps{i}_{b}_{c}",
                        )
                        for i in range(2)
                    ]
                    for t, (dr, dc) in enumerate(TAPS):
                        st = t == 0
                        sp = t == 8
                        base = 1 + (2 * c + dr) * w + dc - 1
                        for img in range(IMGS_PER_CORE):
                            p0 = img * 64
                            rhs = bt[p0 : p0 + 64, base : base + 2 * w].rearrange(
                                "p (a c) -> p a c", c=w
                            )
                            nc.tensor.matmul(
                                ps[img][:],
                                w_sb[p0 : p0 + 64, t, :],
                                rhs,
                                start=st,
                                stop=sp,
                            )
                    for img in range(IMGS_PER_CORE):
                        dst = ost[img][:, c * chunk : (c + 1) * chunk]
                        if c == n_chunk - 1:
                            # split the final pair across engines so the last
                            # evictions (gating the tail stores) parallelize
                            if img == 0:
                                nc.vector.tensor_scalar_add(dst, ps[img][:], bias_sb[:])
                            else:
                                nc.scalar.add(dst, ps[img][:], bias_sb[:])
                        elif (c % 4) < evict_split:
                            nc.vector.tensor_scalar_add(dst, ps[img][:], bias_sb[:])
                        else:
                            nc.scalar.add(dst, ps[img][:], bias_sb[:])
                    if c == 0 and b > 0:
                        # after chunk 0 so the PE's in-order queue reaches
                        # these only once the previous band's strip readers
                        # (its patches) have long finished -- no WAR stall.
                        strips = emit_fix()

                if strips is None:
                    strips = emit_fix()

                # Patch borders and store. On the last band, patch/store in
                # row quarters so the stores pipeline with the trailing
                # evictions instead of draining ~4MB after the final matmul.
                n_out = 8 if b == n_bands - 1 else 2
                for oc in range(n_out):
                    r_a = oc * r // n_out
                    r_b = (oc + 1) * r // n_out
                    for img in range(IMGS_PER_CORE):
                        nc.vector.tensor_add(
                            ost[img][:, r_a * w : r_b * w : w],
                            ost[img][:, r_a * w : r_b * w : w],
                            strips[img][:, r_a:r_b],
                        )
                        nc.vector.tensor_add(
                            ost[img][:, r_a * w + w - 1 : r_b * w : w],
                            ost[img][:, r_a * w + w - 1 : r_b * w : w],
                            strips[img][:, r + r_a : r + r_b],
                        )
                        nc.scalar.dma_start(
                            out=out_d[img, :, y0 + r_a : y0 + r_b, :],
                            in_=ost[img][:, r_a * w : r_b * w],
                        )

    nc.compile()
    return nc


def prep_weight(weight: np.ndarray) -> np.ndarray:
    # [C_OUT, C_IN, 3, 3] -> [C_IN, 9, C_OUT]
    return np.ascontiguousarray(weight.transpose(1, 2, 3, 0).reshape(C_IN, 9, C_OUT))


def run_conv(x, weight, bias, trace=False, h=H, r=R, out_bf16=True, evict_split=3):
    """x [16,64,224,224] f32. Returns (out [16,128,224,224] f32, results)."""
    from concourse.bass_utils import run_bass_kernel_spmd

    x = np.asarray(x, dtype=np.float32).astype(np.float16)
    w_t = prep_weight(np.asarray(weight, dtype=np.float32)).astype(np.float16)
    # negated dc=0 taps (indices 0,3,6) then negated dc=2 taps (2,5,8)
    wn_t = np.ascontiguousarray(
        np.concatenate([-w_t[:, 0::3, :], -w_t[:, 2::3, :]], axis=1)
    )
    b_t = np.ascontiguousarray(np.asarray(bias, dtype=np.float32).reshape(C_OUT, 1))

    nc = build_conv_program(h=h, r=r, out_bf16=out_bf16, evict_split=evict_split)
    in_maps = [
        {
            "x": np.ascontiguousarray(x[i * IMGS_PER_CORE : (i + 1) * IMGS_PER_CORE]),
            "w": w_t,
            "wneg": wn_t,
            "bias": b_t,
        }
        for i in range(N_CORES)
    ]
    res = run_bass_kernel_spmd(nc, in_maps, core_ids=list(range(N_CORES)), trace=trace)
    out = np.concatenate([r_["out"] for r_ in res.results], axis=0)
    if out.dtype != np.float32:
        out = out.astype(np.float32)
    return out, res


def kernel(**inputs) -> np.ndarray:
    out, _ = run_conv(inputs["x"], inputs["weight"], inputs["bias"])
    return out
